# revision 49
# baseline (speedup 1.0000x reference)
"""Causal self-attention (B=2, T=2048, C=1024, H=16) on 8 trn2 NeuronCores.

Sharding: core = b*4 + g  ->  batch b, heads 4g..4g+3 (tensor-parallel on the
head/C dimension of the QKV and output projections).  Each core computes full-T
causal attention for its 4 heads and a partial output projection; the host sums
the 4 partials per batch and adds bo.

Dataflow (q-block j of 512 rows, head-pair p, k-tile of 128):
  S^T = K Q^T per head as two row-tiled matmuls (tile_position (64e, 0)) into a
  [128, 1024] PSUM tile; diagonal k-tiles restrict S to columns >= the tile's
  first open q and the mask multiply zeroes the stale remainder.
  V carries an appended ones-column per head (65 columns), so each PV matmul
  (M=65, output partitions 0..64) yields O' in rows 0..63 and the softmax
  denominator in row 64 - no separate denominator matmul.
  j = 0 (small softmax support, kept bf16 to protect accuracy): ACT exp ->
  bf16 P; PV as plain bf16 matmuls.
  j >= 1 (fp8 fast path): ACT exp -> fp8e4 P written into pair-slot s of a
  [128, 2, 1024] tile spanning TWO adjacent k-tiles; PV is an fp8 DoubleRow
  matmul contracting 256 keys per instruction (128 partitions x 2 pair slots)
  at 0.5 cycles/row - 3x fewer PE cycles than the bf16 path per key.
  Normalize: denominator row -> SBUF via DMA -> K=1 f32r broadcast matmul
  replicates it across partitions (head e -> rows 64e..64e+63); head-1 O' is
  partition-shifted by DMA; fast reciprocal + bf16 multiply build onorm;
  ypart[T, C] = onorm^T.T @ Wo (bf16) accumulated over two 128-row chunks.
"""

import numpy as np
import ml_dtypes

import concourse.bass as bass
import concourse.mybir as mybir
import concourse.tile as tile
from concourse.tile import add_dep_helper
from concourse import bacc
from concourse.bass_utils import run_bass_kernel_spmd
from concourse.dve_ops import RECIPROCAL_APPROX_FAST, RECIP_APPROX_FAST_CONSTS

B, T, C, H, D = 2, 2048, 1024, 16, 64
N_CORES = 8
HS = 256              # head-dim slice per core (4 heads x 64)
NQ = T // 512         # 4 q-tiles of 512
NK = T // 128         # 16 k-tiles of 128
NP = NK // 2          # 8 k-tile pairs (fp8 DoubleRow granularity)
NC8 = C // 128        # 8 contraction chunks
F32 = mybir.dt.float32
F32R = mybir.dt.float32r
BF16 = mybir.dt.bfloat16
FP8 = mybir.dt.float8e4
DR = mybir.MatmulPerfMode.DoubleRow

_CACHE = {}


def _r(ap):
    return ap.bitcast(F32R)


def _build():
    nc = bacc.Bacc("TRN2", target_bir_lowering=False, debug=False,
                   num_devices=N_CORES)

    # all big inputs arrive pre-arranged on the host as SBUF images so each
    # is one (or a few) wide fully-contiguous DMA
    xt_d = nc.dram_tensor("xt", [128, NC8 * T], BF16, kind="ExternalInput")
    wq_d = nc.dram_tensor("wq", [128, NC8 * HS], BF16, kind="ExternalInput")
    wk_d = nc.dram_tensor("wk", [128, NC8 * HS], BF16, kind="ExternalInput")
    wv_d = nc.dram_tensor("wv", [128, NC8 * HS], BF16, kind="ExternalInput")
    wo_d = nc.dram_tensor("wo", [128, 2 * C], BF16, kind="ExternalInput")
    bq_d = nc.dram_tensor("bq", [128, 2], F32, kind="ExternalInput")
    bk_d = nc.dram_tensor("bk", [128, 2], F32, kind="ExternalInput")
    bv_d = nc.dram_tensor("bv", [128, 4, 64], F32, kind="ExternalInput")
    mo_d = nc.dram_tensor("mo", [128, 896], BF16, kind="ExternalInput")
    mn_d = nc.dram_tensor("mn", [128, 896], FP8, kind="ExternalInput")
    y_d = nc.dram_tensor("y", [T, C], F32, kind="ExternalOutput")

    with tile.TileContext(nc) as tc:
        with (
            tc.tile_pool(name="const", bufs=1) as cpool,
            tc.tile_pool(name="pp0", bufs=2) as p0pool,
            tc.tile_pool(name="pp8", bufs=4) as p8pool,
            tc.tile_pool(name="onorm", bufs=4) as opool,
            tc.tile_pool(name="bc", bufs=2) as bcpool,
            tc.tile_pool(name="outp", bufs=4) as outpool,
            tc.tile_pool(name="spsum", bufs=2, space="PSUM") as spool,
            tc.tile_pool(name="opsum", bufs=1, space="PSUM") as oppool,
            tc.tile_pool(name="nyps", bufs=2, space="PSUM") as gpool,
        ):
            # ---- persistent SBUF tensors ----
            xt_s = cpool.tile([128, NC8 * T], BF16, tag="xt")
            wq_s = cpool.tile([128, NC8 * HS], BF16, tag="wq")
            wk_s = cpool.tile([128, NC8 * HS], BF16, tag="wk")
            wv_s = cpool.tile([128, NC8 * HS], BF16, tag="wv")
            wo_s = cpool.tile([128, 2 * C], BF16, tag="wo")
            v_s = cpool.tile([128, 4, 4, 128], BF16, tag="vs")
            v8_s = cpool.tile([128, NP, 2, 4, 128], FP8, tag="v8")
            qt_s = [cpool.tile([128, T], BF16, tag=f"qt{p}", name=f"qt{p}")
                    for p in range(2)]
            kt_s = [cpool.tile([128, T], BF16, tag=f"kt{p}", name=f"kt{p}")
                    for p in range(2)]
            mo_s = cpool.tile([128, 896], BF16, tag="mo")
            mn_s = cpool.tile([128, 896], FP8, tag="mn")
            bq_s = cpool.tile([128, 2], F32, tag="bq")
            bk_s = cpool.tile([128, 2], F32, tag="bk")
            bv_s = cpool.tile([128, 4, 64], F32, tag="bv")

            # ones half-planes of the augmented V tensors: each PV matmul
            # then emits the replicated denominator in rows 0..63 and O'
            # in rows 64..127
            for t in range(4):
                nc.vector.memset(v_s[:, t, :, 0:64], 1.0)
            for a in range(NP):
                for s in range(2):
                    nc.vector.memset(v8_s[:, a, s, :, 0:64], 1.0)

            # ---- input DMAs: smalls + masks + weights, then xt as one
            # contiguous [128, 4096] slice per q/k-range n, wo last ----
            nc.sync.dma_start(out=bq_s[:], in_=bq_d.ap())
            nc.sync.dma_start(out=bk_s[:], in_=bk_d.ap())
            nc.sync.dma_start(out=bv_s[:], in_=bv_d.ap())
            nc.sync.dma_start(out=mo_s[:], in_=mo_d.ap())
            nc.sync.dma_start(out=mn_s[:], in_=mn_d.ap())
            for w_s, w_d in ((wq_s, wq_d), (wk_s, wk_d), (wv_s, wv_d)):
                nc.sync.dma_start(out=w_s[:], in_=w_d.ap())
            nc.sync.dma_start(out=xt_s[:, 0:4096], in_=xt_d.ap()[:, 0:4096])
            # later inputs go on the Activation HWDGE queue so the first
            # matmuls only wait on the sync-queue DMAs above
            for n in range(1, NQ):
                nc.scalar.dma_start(
                    out=xt_s[:, 4096 * n:4096 * (n + 1)],
                    in_=xt_d.ap()[:, 4096 * n:4096 * (n + 1)])
            nc.scalar.dma_start(out=wo_s[:], in_=wo_d.ap())

            def qkv_group_thunks(n):
                """Per-group emission thunks for QT/KT/V of q/k-range n."""
                thunks = []
                for p in range(2):
                    for w_s, b_s, t_s in ((wq_s, bq_s, qt_s), (wk_s, bk_s, kt_s)):
                        def th(p=p, w_s=w_s, b_s=b_s, t_s=t_s):
                            ps = gpool.tile([128, 512], F32, tag="g")
                            for c in range(NC8):
                                nc.tensor.matmul(
                                    ps[:],
                                    w_s[:, HS * c + 128 * p:
                                        HS * c + 128 * (p + 1)],
                                    xt_s[:, 4096 * n + 512 * c:
                                         4096 * n + 512 * (c + 1)],
                                    start=(c == 0), stop=(c == NC8 - 1))
                            nc.vector.tensor_scalar_add(
                                out=t_s[p][:, 512 * n:512 * (n + 1)],
                                in0=ps[:], scalar1=b_s[:, p:p + 1])
                        thunks.append(th)
                for u in range(4):
                    def th(u=u):
                        t_idx = 4 * n + u
                        ps = gpool.tile([128, 4, 64], F32, tag="g",
                                        name=f"vps_{t_idx}")
                        xb = 4096 * (t_idx // 4) + 128 * (t_idx % 4)
                        for c in range(NC8):
                            nc.tensor.matmul(
                                ps[:],
                                xt_s[:, xb + 512 * c:xb + 512 * c + 128],
                                wv_s[:, HS * c:HS * (c + 1)],
                                start=(c == 0), stop=(c == NC8 - 1))
                        # fp8 V (pair-slot layout) for the DoubleRow path
                        nc.vector.tensor_add(
                            out=v8_s[:, t_idx // 2, t_idx % 2, :, 64:128],
                            in0=ps[:], in1=bv_s[:])
                        if t_idx < 4:
                            # bf16 V for the j=0 block
                            nc.vector.tensor_add(
                                out=v_s[:, t_idx, :, 64:128],
                                in0=ps[:], in1=bv_s[:])
                    thunks.append(th)
                return thunks

            def proj_group_thunks(j, onorm):
                thunks = []
                for u in range(4):
                    for n2 in range(2):
                        def th(u=u, n2=n2):
                            y_ps = gpool.tile([128, 512], F32, tag="g")
                            for p in range(2):
                                nc.tensor.matmul(
                                    y_ps[:],
                                    onorm[p][:, 128 * u:128 * (u + 1)],
                                    wo_s[:, C * p + 512 * n2:
                                         C * p + 512 * (n2 + 1)],
                                    start=(p == 0), stop=(p == 1))
                            out_t = outpool.tile([128, 512], F32, tag="out")
                            nc.vector.tensor_copy(out_t[:], y_ps[:])
                            nc.sync.dma_start(
                                out=y_d.ap()[512 * j + 128 * u:
                                             512 * j + 128 * (u + 1),
                                             512 * n2:512 * (n2 + 1)],
                                in_=out_t[:])
                        thunks.append(th)
                return thunks

            filler = []
            fill_credit = [0.0]

            def emit_filler(credit):
                """Accumulate fractional credit; emit whole thunks evenly."""
                fill_credit[0] += credit
                while fill_credit[0] >= 1.0 and filler:
                    fill_credit[0] -= 1.0
                    filler.pop(0)()

            def flush_filler():
                fill_credit[0] = 0.0
                for _ in range(len(filler)):
                    filler.pop(0)()

            def normalize(j, p, o8):
                """o8 [128, 2, 512]: per head e, rows 0..63 hold the
                replicated denominator and rows 64..127 hold O'.

                Returns the bf16 onorm tile [128, 512] (head e on rows
                64e..64e+63), denominator-normalized.  Partition shifts
                (head stacking) go through SBUF->SBUF DMAs.
                """
                stg = opool.tile([128, 2, 512], F32, tag="stg",
                                 name=f"stg_{j}_{p}")
                nc.vector.tensor_copy(stg[:], o8[:])
                dnm = bcpool.tile([128, 512], F32, tag="dnm",
                                  name=f"dnm_{j}_{p}")
                nc.sync.dma_start(out=dnm[0:64, :], in_=stg[0:64, 0, :])
                nc.sync.dma_start(out=dnm[64:128, :], in_=stg[0:64, 1, :])
                ot_f = opool.tile([128, 512], F32, tag="onf",
                                  name=f"onf_{j}_{p}")
                nc.sync.dma_start(out=ot_f[0:64, :], in_=stg[64:128, 0, :])
                rr = bcpool.tile([128, 512], F32, tag="rr",
                                 name=f"rr_{j}_{p}")
                nc.vector._custom_dve(
                    RECIPROCAL_APPROX_FAST, out=rr[:], in0=dnm[:],
                    s0=RECIP_APPROX_FAST_CONSTS["s0"],
                    s1=RECIP_APPROX_FAST_CONSTS["s1"],
                    imm2=RECIP_APPROX_FAST_CONSTS["imm2"])
                ot = opool.tile([128, 512], BF16, tag="onorm",
                                name=f"onorm_{j}_{p}")
                nc.vector.tensor_mul(ot[0:64, :], ot_f[0:64, :],
                                     rr[0:64, :])
                nc.vector.tensor_mul(ot[64:128, :], stg[64:128, 1, :],
                                     rr[64:128, :])
                return ot

            s_seed = [0]
            s_c0 = [0]

            def emit_s_tile(j, p, k, s_ps):
                """S^T for k-tile k of q-block j: two row-tiled matmuls.

                Diagonal tiles restrict columns to [128m, 512); the mask
                multiply later zeroes the stale prefix.  The first two S
                tiles (one per rotating PSUM buffer) run full-width so no
                tile ever exposes pre-kernel PSUM garbage to exp (whose
                inf would turn the mask multiply into NaN).
                """
                m = k - 4 * j
                c0 = 128 * m if m >= 0 and s_seed[0] >= 2 else 0
                s_seed[0] += 1
                s_c0[0] = c0
                for e in range(2):
                    nc.tensor.matmul(
                        s_ps[:, 512 * e + c0:512 * (e + 1)],
                        kt_s[p][64 * e:64 * (e + 1),
                                128 * k:128 * (k + 1)],
                        qt_s[p][64 * e:64 * (e + 1),
                                512 * j + c0:512 * (j + 1)],
                        start=True, stop=True,
                        tile_position=(64 * e, 0))
                return m

            def emit_attention_j0():
                """q-block 0: all-diagonal, bf16 PV."""
                per_iter = len(filler) / 8.0
                onorm = []
                for p in range(2):
                    o8 = oppool.tile([128, 2, 512], F32, tag="o",
                                     name=f"o8_{p}_0")
                    for k in range(4):
                        s_ps = spool.tile([128, 1024], F32, tag="s",
                                          name=f"s_0_{k}_{p}")
                        m = emit_s_tile(0, p, k, s_ps)
                        pt = p0pool.tile([128, 1024], BF16, tag="p0",
                                         name=f"p_0_{k}_{p}")
                        c0 = s_c0[0]
                        if c0 == 0:
                            nc.scalar.activation(
                                pt[:], s_ps[:],
                                mybir.ActivationFunctionType.Exp)
                        else:
                            for e in range(2):
                                nc.scalar.activation(
                                    pt[:, 512 * e + c0:512 * (e + 1)],
                                    s_ps[:, 512 * e + c0:512 * (e + 1)],
                                    mybir.ActivationFunctionType.Exp)
                        w = 128 * m + 128
                        for e in range(2):
                            nc.vector.tensor_mul(
                                pt[:, 512 * e:512 * e + w],
                                pt[:, 512 * e:512 * e + w],
                                mo_s[:, 896 - w:896])
                        for e in range(2):
                            nc.tensor.matmul(
                                o8[:, e, :],
                                v_s[:, k, 2 * p + e, :],
                                pt[:, 512 * e:512 * (e + 1)],
                                start=(k == 0), stop=(k == 3),
                                tile_position=(0, 0))
                        emit_filler(per_iter)
                    onorm.append(normalize(0, p, o8))
                return onorm

            def emit_attention_fp8(j):
                """q-block j>=1: fp8 DoubleRow PV per k-tile pair."""
                nk = 4 * (j + 1)
                na = nk // 2
                per_iter = len(filler) / (2.0 * nk)
                onorm = []
                if not hasattr(emit_attention_fp8, "seed"):
                    emit_attention_fp8.seed = [0]
                p8_seed = emit_attention_fp8.seed
                for p in range(2):
                    o8 = oppool.tile([128, 2, 512], F32, tag="o",
                                     name=f"o8_{p}_{j}")
                    for a in range(na):
                        pt = p8pool.tile([128, 2, 1024], FP8, tag="p8",
                                         name=f"p_{j}_{a}_{p}")
                        pt_virgin = p8_seed[0] < 4
                        p8_seed[0] += 1
                        for s in range(2):
                            k = 2 * a + s
                            s_ps = spool.tile([128, 1024], F32, tag="s",
                                              name=f"s_{j}_{k}_{p}")
                            m = emit_s_tile(j, p, k, s_ps)
                            c0 = 0 if pt_virgin else s_c0[0]
                            if c0 == 0:
                                nc.scalar.activation(
                                    pt[:, s, :], s_ps[:],
                                    mybir.ActivationFunctionType.Exp)
                            else:
                                for e in range(2):
                                    nc.scalar.activation(
                                        pt[:, s, 512 * e + c0:512 * (e + 1)],
                                        s_ps[:, 512 * e + c0:512 * (e + 1)],
                                        mybir.ActivationFunctionType.Exp)
                            if m >= 0:
                                w = 128 * m + 128
                                for e in range(2):
                                    nc.vector.tensor_mul(
                                        pt[:, s, 512 * e:512 * e + w],
                                        pt[:, s, 512 * e:512 * e + w],
                                        mn_s[:, 896 - w:896])
                            emit_filler(per_iter)
                        for e in range(2):
                            nc.tensor.matmul(
                                o8[:, e, :],
                                v8_s[:, a, :, 2 * p + e, :],
                                pt[:, :, 512 * e:512 * (e + 1)],
                                start=(a == 0), stop=(a == na - 1),
                                perf_mode=DR,
                                tile_position=(0, 0))
                    onorm.append(normalize(j, p, o8))
                return onorm

            for th in qkv_group_thunks(0):
                th()
            for j in range(NQ):
                filler.extend(qkv_group_thunks(j + 1) if j + 1 < NQ else [])
                onorm = emit_attention_j0() if j == 0 else emit_attention_fp8(j)
                filler.extend(proj_group_thunks(j, onorm))
            flush_filler()

    nc.compile()
    return nc


def _get_nc():
    if "nc" not in _CACHE:
        _CACHE["nc"] = _build()
    return _CACHE["nc"]


def _masks():
    """Master causal masks, sliced per diagonal tile at emission time.

    mo/mn [128, 896]: mask[kk, z] = (z >= 768 + kk).  The w-wide suffix
    slice gives [stale-zero prefix | 128-col triangle] for tile offset m:
    col >= 128m + kk  <=>  z = col + 896 - w >= 768 + kk  (w = 128m+128).
    """
    if "mo" not in _CACHE:
        kk = np.arange(128)[:, None]
        z = np.arange(896)[None, :]
        m = (z >= 768 + kk)
        _CACHE["mo"] = m.astype(ml_dtypes.bfloat16)
        _CACHE["mn"] = m.astype(ml_dtypes.float8_e4m3)
    return _CACHE["mo"], _CACHE["mn"]


def kernel(x, Wq, bq, Wk, bk, Wv, bv, Wo, bo):
    x = np.asarray(x, np.float32)
    Wq, bq = np.asarray(Wq, np.float32), np.asarray(bq, np.float32)
    Wk, bk = np.asarray(Wk, np.float32), np.asarray(bk, np.float32)
    Wv, bv = np.asarray(Wv, np.float32), np.asarray(bv, np.float32)
    Wo, bo = np.asarray(Wo, np.float32), np.asarray(bo, np.float32)

    nc = _get_nc()
    mo, mn = _masks()

    def _wimg(w):  # [1024, 256] -> SBUF image [128, 8*256]
        return np.ascontiguousarray(
            w.reshape(8, 128, HS).transpose(1, 0, 2).reshape(128, 8 * HS)
        ).astype(ml_dtypes.bfloat16)

    in_maps = []
    for core in range(N_CORES):
        b, g = divmod(core, 4)
        sl = slice(HS * g, HS * (g + 1))
        xt = x[b].T.reshape(8, 128, 4, 512).transpose(1, 2, 0, 3)
        wo = Wo[sl, :].reshape(2, 128, C).transpose(1, 0, 2)
        in_maps.append({
            "xt": np.ascontiguousarray(xt.reshape(128, NC8 * T)
                                       ).astype(ml_dtypes.bfloat16),
            "wq": _wimg(Wq[:, sl] * 0.125),
            "wk": _wimg(Wk[:, sl]),
            "wv": _wimg(Wv[:, sl]),
            "wo": np.ascontiguousarray(wo.reshape(128, 2 * C)
                                       ).astype(ml_dtypes.bfloat16),
            "bq": (bq[sl] * 0.125).reshape(2, 128).T.copy(),
            "bk": bk[sl].reshape(2, 128).T.copy(),
            "bv": np.broadcast_to(bv[sl], (128, HS)).reshape(128, 4, 64).copy(),
            "mo": mo,
            "mn": mn,
        })

    res = run_bass_kernel_spmd(nc, in_maps, core_ids=list(range(N_CORES)),
                               **_CACHE.get("run_kwargs", {}))
    _CACHE["last_result"] = res

    y = np.zeros((B, T, C), np.float32)
    for core in range(N_CORES):
        b = core // 4
        y[b] += res.results[core]["y"]
    y += bo
    return y


# revision 52
# speedup vs baseline: 1.0497x; 1.0497x over previous
"""Causal self-attention (B=2, T=2048, C=1024, H=16) on 8 trn2 NeuronCores.

Sharding: core = b*4 + g  ->  batch b, heads 4g..4g+3 (tensor-parallel on the
head/C dimension of the QKV and output projections).  Each core computes full-T
causal attention for its 4 heads and a partial output projection; the host sums
the 4 partials per batch and adds bo.

Dataflow (q-block j of 512 rows, head-pair p, k-tile of 128):
  S^T = K Q^T per head as two row-tiled matmuls (tile_position (64e, 0)) into a
  [128, 1024] PSUM tile; diagonal k-tiles restrict S to columns >= the tile's
  first open q and the mask multiply zeroes the stale remainder.
  V carries an appended ones-column per head (65 columns), so each PV matmul
  (M=65, output partitions 0..64) yields O' in rows 0..63 and the softmax
  denominator in row 64 - no separate denominator matmul.
  j = 0 (small softmax support, kept bf16 to protect accuracy): ACT exp ->
  bf16 P; PV as plain bf16 matmuls.
  j >= 1 (fp8 fast path): ACT exp -> fp8e4 P written into pair-slot s of a
  [128, 2, 1024] tile spanning TWO adjacent k-tiles; PV is an fp8 DoubleRow
  matmul contracting 256 keys per instruction (128 partitions x 2 pair slots)
  at 0.5 cycles/row - 3x fewer PE cycles than the bf16 path per key.
  Normalize: denominator row -> SBUF via DMA -> K=1 f32r broadcast matmul
  replicates it across partitions (head e -> rows 64e..64e+63); head-1 O' is
  partition-shifted by DMA; fast reciprocal + bf16 multiply build onorm;
  ypart[T, C] = onorm^T.T @ Wo (bf16) accumulated over two 128-row chunks.
"""

import numpy as np
import ml_dtypes

import concourse.bass as bass
import concourse.mybir as mybir
import concourse.tile as tile
from concourse.tile import add_dep_helper
from concourse import bacc
from concourse.bass_utils import run_bass_kernel_spmd
from concourse.dve_ops import RECIPROCAL_APPROX_FAST, RECIP_APPROX_FAST_CONSTS

B, T, C, H, D = 2, 2048, 1024, 16, 64
N_CORES = 8
HS = 256              # head-dim slice per core (4 heads x 64)
NQ = T // 512         # 4 q-tiles of 512
NK = T // 128         # 16 k-tiles of 128
NP = NK // 2          # 8 k-tile pairs (fp8 DoubleRow granularity)
NC8 = C // 128        # 8 contraction chunks
F32 = mybir.dt.float32
F32R = mybir.dt.float32r
BF16 = mybir.dt.bfloat16
FP8 = mybir.dt.float8e4
DR = mybir.MatmulPerfMode.DoubleRow

_CACHE = {}


def _r(ap):
    return ap.bitcast(F32R)


def _build():
    nc = bacc.Bacc("TRN2", target_bir_lowering=False, debug=False,
                   num_devices=N_CORES)

    # all big inputs arrive pre-arranged on the host as SBUF images so each
    # is one (or a few) wide fully-contiguous DMA
    xt_d = nc.dram_tensor("xt", [128, NC8 * T], BF16, kind="ExternalInput")
    wq_d = nc.dram_tensor("wq", [128, NC8 * HS], BF16, kind="ExternalInput")
    wk_d = nc.dram_tensor("wk", [128, NC8 * HS], BF16, kind="ExternalInput")
    wv_d = nc.dram_tensor("wv", [128, NC8 * HS], BF16, kind="ExternalInput")
    wo_d = nc.dram_tensor("wo", [128, 2 * C], BF16, kind="ExternalInput")
    bq_d = nc.dram_tensor("bq", [128, 2], F32, kind="ExternalInput")
    bk_d = nc.dram_tensor("bk", [128, 2], F32, kind="ExternalInput")
    bv_d = nc.dram_tensor("bv", [128, 4, 64], F32, kind="ExternalInput")
    mo_d = nc.dram_tensor("mo", [128, 896], BF16, kind="ExternalInput")
    mn_d = nc.dram_tensor("mn", [128, 896], FP8, kind="ExternalInput")
    y_d = nc.dram_tensor("y", [T, C], F32, kind="ExternalOutput")

    with tile.TileContext(nc) as tc:
        with (
            tc.tile_pool(name="const", bufs=1) as cpool,
            tc.tile_pool(name="pp0", bufs=2) as p0pool,
            tc.tile_pool(name="pp8", bufs=4) as p8pool,
            tc.tile_pool(name="onorm", bufs=4) as opool,
            tc.tile_pool(name="bc", bufs=2) as bcpool,
            tc.tile_pool(name="outp", bufs=4) as outpool,
            tc.tile_pool(name="spsum", bufs=2, space="PSUM") as spool,
            tc.tile_pool(name="opsum", bufs=1, space="PSUM") as oppool,
            tc.tile_pool(name="nyps", bufs=2, space="PSUM") as gpool,
        ):
            # ---- persistent SBUF tensors ----
            xt_s = cpool.tile([128, NC8 * T], BF16, tag="xt")
            wq_s = cpool.tile([128, NC8 * HS], BF16, tag="wq")
            wk_s = cpool.tile([128, NC8 * HS], BF16, tag="wk")
            wv_s = cpool.tile([128, NC8 * HS], BF16, tag="wv")
            wo_s = cpool.tile([128, 2 * C], BF16, tag="wo")
            v_s = cpool.tile([128, 4, 4, 128], BF16, tag="vs")
            v8_s = cpool.tile([128, NP, 2, 4, 128], FP8, tag="v8")
            qt_s = [cpool.tile([128, T], BF16, tag=f"qt{p}", name=f"qt{p}")
                    for p in range(2)]
            kt_s = [cpool.tile([128, T], BF16, tag=f"kt{p}", name=f"kt{p}")
                    for p in range(2)]
            mo_s = cpool.tile([128, 896], BF16, tag="mo")
            mn_s = cpool.tile([128, 896], FP8, tag="mn")
            bq_s = cpool.tile([128, 2], F32, tag="bq")
            bk_s = cpool.tile([128, 2], F32, tag="bk")
            bv_s = cpool.tile([128, 4, 64], F32, tag="bv")

            # ones half-planes of the augmented V tensors: each PV matmul
            # then emits the replicated denominator in rows 0..63 and O'
            # in rows 64..127
            for t in range(4):
                nc.vector.memset(v_s[:, t, :, 0:64], 1.0)
            for a in range(NP):
                for s in range(2):
                    nc.vector.memset(v8_s[:, a, s, :, 0:64], 1.0)

            # ---- input DMAs: smalls + masks + weights, then xt as one
            # contiguous [128, 4096] slice per q/k-range n, wo last ----
            nc.sync.dma_start(out=bq_s[:], in_=bq_d.ap())
            nc.sync.dma_start(out=bk_s[:], in_=bk_d.ap())
            nc.sync.dma_start(out=bv_s[:], in_=bv_d.ap())
            nc.sync.dma_start(out=mo_s[:], in_=mo_d.ap())
            nc.sync.dma_start(out=mn_s[:], in_=mn_d.ap())
            for w_s, w_d in ((wq_s, wq_d), (wk_s, wk_d), (wv_s, wv_d)):
                nc.sync.dma_start(out=w_s[:], in_=w_d.ap())
            nc.sync.dma_start(out=xt_s[:, 0:4096], in_=xt_d.ap()[:, 0:4096])

            first_mm = []

            def emit_late_dmas():
                """Bulk inputs not needed by the first QKV group, gated
                behind the first matmul so they don't steal DMA bandwidth
                from the startup-critical transfers above."""
                late = [nc.sync.dma_start(
                    out=xt_s[:, 4096 * n:4096 * (n + 1)],
                    in_=xt_d.ap()[:, 4096 * n:4096 * (n + 1)])
                    for n in range(1, NQ)]
                late.append(nc.sync.dma_start(out=wo_s[:], in_=wo_d.ap()))
                for dd in late:
                    add_dep_helper(dd.ins, first_mm[0].ins,
                                   reason="defer bulk input DMA")

            def qkv_group_thunks(n):
                """Per-group emission thunks for QT/KT/V of q/k-range n."""
                thunks = []
                for p in range(2):
                    for w_s, b_s, t_s in ((wq_s, bq_s, qt_s), (wk_s, bk_s, kt_s)):
                        def th(p=p, w_s=w_s, b_s=b_s, t_s=t_s):
                            ps = gpool.tile([128, 512], F32, tag="g")
                            for c in range(NC8):
                                mm = nc.tensor.matmul(
                                    ps[:],
                                    w_s[:, HS * c + 128 * p:
                                        HS * c + 128 * (p + 1)],
                                    xt_s[:, 4096 * n + 512 * c:
                                         4096 * n + 512 * (c + 1)],
                                    start=(c == 0), stop=(c == NC8 - 1))
                                if not first_mm:
                                    first_mm.append(mm)
                            nc.vector.tensor_scalar_add(
                                out=t_s[p][:, 512 * n:512 * (n + 1)],
                                in0=ps[:], scalar1=b_s[:, p:p + 1])
                        thunks.append(th)
                for u in range(4):
                    def th(u=u):
                        t_idx = 4 * n + u
                        ps = gpool.tile([128, 4, 64], F32, tag="g",
                                        name=f"vps_{t_idx}")
                        xb = 4096 * (t_idx // 4) + 128 * (t_idx % 4)
                        for c in range(NC8):
                            nc.tensor.matmul(
                                ps[:],
                                xt_s[:, xb + 512 * c:xb + 512 * c + 128],
                                wv_s[:, HS * c:HS * (c + 1)],
                                start=(c == 0), stop=(c == NC8 - 1))
                        # fp8 V (pair-slot layout) for the DoubleRow path
                        nc.vector.tensor_add(
                            out=v8_s[:, t_idx // 2, t_idx % 2, :, 64:128],
                            in0=ps[:], in1=bv_s[:])
                        if t_idx < 4:
                            # bf16 V for the j=0 block
                            nc.vector.tensor_add(
                                out=v_s[:, t_idx, :, 64:128],
                                in0=ps[:], in1=bv_s[:])
                    thunks.append(th)
                return thunks

            def proj_group_thunks(j, onorm):
                thunks = []
                for u in range(4):
                    for n2 in range(2):
                        def th(u=u, n2=n2):
                            y_ps = gpool.tile([128, 512], F32, tag="g")
                            for p in range(2):
                                nc.tensor.matmul(
                                    y_ps[:],
                                    onorm[p][:, 128 * u:128 * (u + 1)],
                                    wo_s[:, C * p + 512 * n2:
                                         C * p + 512 * (n2 + 1)],
                                    start=(p == 0), stop=(p == 1))
                            out_t = outpool.tile([128, 512], F32, tag="out")
                            nc.vector.tensor_copy(out_t[:], y_ps[:])
                            nc.sync.dma_start(
                                out=y_d.ap()[512 * j + 128 * u:
                                             512 * j + 128 * (u + 1),
                                             512 * n2:512 * (n2 + 1)],
                                in_=out_t[:])
                        thunks.append(th)
                return thunks

            filler = []
            fill_credit = [0.0]

            def emit_filler(credit):
                """Accumulate fractional credit; emit whole thunks evenly."""
                fill_credit[0] += credit
                while fill_credit[0] >= 1.0 and filler:
                    fill_credit[0] -= 1.0
                    filler.pop(0)()

            def flush_filler():
                fill_credit[0] = 0.0
                for _ in range(len(filler)):
                    filler.pop(0)()

            def normalize(j, p, o8):
                """o8 [128, 2, 512]: per head e, rows 0..63 hold the
                replicated denominator and rows 64..127 hold O'.

                Returns the bf16 onorm tile [128, 512] (head e on rows
                64e..64e+63), denominator-normalized.  Partition shifts
                (head stacking) go through SBUF->SBUF DMAs.
                """
                stg = opool.tile([128, 2, 512], F32, tag="stg",
                                 name=f"stg_{j}_{p}")
                nc.vector.tensor_copy(stg[:], o8[:])
                dnm = bcpool.tile([128, 512], F32, tag="dnm",
                                  name=f"dnm_{j}_{p}")
                nc.sync.dma_start(out=dnm[0:64, :], in_=stg[0:64, 0, :])
                nc.sync.dma_start(out=dnm[64:128, :], in_=stg[0:64, 1, :])
                ot_f = opool.tile([128, 512], F32, tag="onf",
                                  name=f"onf_{j}_{p}")
                nc.sync.dma_start(out=ot_f[0:64, :], in_=stg[64:128, 0, :])
                rr = bcpool.tile([128, 512], F32, tag="rr",
                                 name=f"rr_{j}_{p}")
                nc.vector._custom_dve(
                    RECIPROCAL_APPROX_FAST, out=rr[:], in0=dnm[:],
                    s0=RECIP_APPROX_FAST_CONSTS["s0"],
                    s1=RECIP_APPROX_FAST_CONSTS["s1"],
                    imm2=RECIP_APPROX_FAST_CONSTS["imm2"])
                ot = opool.tile([128, 512], BF16, tag="onorm",
                                name=f"onorm_{j}_{p}")
                nc.vector.tensor_mul(ot[0:64, :], ot_f[0:64, :],
                                     rr[0:64, :])
                nc.vector.tensor_mul(ot[64:128, :], stg[64:128, 1, :],
                                     rr[64:128, :])
                return ot

            s_seed = [0]
            s_c0 = [0]

            def emit_s_tile(j, p, k, s_ps):
                """S^T for k-tile k of q-block j: two row-tiled matmuls.

                Diagonal tiles restrict columns to [128m, 512); the mask
                multiply later zeroes the stale prefix.  The first two S
                tiles (one per rotating PSUM buffer) run full-width so no
                tile ever exposes pre-kernel PSUM garbage to exp (whose
                inf would turn the mask multiply into NaN).
                """
                m = k - 4 * j
                c0 = 128 * m if m >= 0 and s_seed[0] >= 2 else 0
                s_seed[0] += 1
                s_c0[0] = c0
                for e in range(2):
                    nc.tensor.matmul(
                        s_ps[:, 512 * e + c0:512 * (e + 1)],
                        kt_s[p][64 * e:64 * (e + 1),
                                128 * k:128 * (k + 1)],
                        qt_s[p][64 * e:64 * (e + 1),
                                512 * j + c0:512 * (j + 1)],
                        start=True, stop=True,
                        tile_position=(64 * e, 0))
                return m

            def emit_attention_j0():
                """q-block 0: all-diagonal, bf16 PV."""
                per_iter = len(filler) / 8.0
                onorm = []
                for p in range(2):
                    o8 = oppool.tile([128, 2, 512], F32, tag="o",
                                     name=f"o8_{p}_0")
                    for k in range(4):
                        s_ps = spool.tile([128, 1024], F32, tag="s",
                                          name=f"s_0_{k}_{p}")
                        m = emit_s_tile(0, p, k, s_ps)
                        pt = p0pool.tile([128, 1024], BF16, tag="p0",
                                         name=f"p_0_{k}_{p}")
                        c0 = s_c0[0]
                        if c0 == 0:
                            nc.scalar.activation(
                                pt[:], s_ps[:],
                                mybir.ActivationFunctionType.Exp)
                        else:
                            for e in range(2):
                                nc.scalar.activation(
                                    pt[:, 512 * e + c0:512 * (e + 1)],
                                    s_ps[:, 512 * e + c0:512 * (e + 1)],
                                    mybir.ActivationFunctionType.Exp)
                        w = 128 * m + 128
                        for e in range(2):
                            nc.vector.tensor_mul(
                                pt[:, 512 * e:512 * e + w],
                                pt[:, 512 * e:512 * e + w],
                                mo_s[:, 896 - w:896])
                        for e in range(2):
                            nc.tensor.matmul(
                                o8[:, e, :],
                                v_s[:, k, 2 * p + e, :],
                                pt[:, 512 * e:512 * (e + 1)],
                                start=(k == 0), stop=(k == 3),
                                tile_position=(0, 0))
                        emit_filler(per_iter)
                    onorm.append(normalize(0, p, o8))
                return onorm

            def emit_attention_fp8(j):
                """q-block j>=1: fp8 DoubleRow PV per k-tile pair."""
                nk = 4 * (j + 1)
                na = nk // 2
                per_iter = len(filler) / (2.0 * nk)
                onorm = []
                if not hasattr(emit_attention_fp8, "seed"):
                    emit_attention_fp8.seed = [0]
                p8_seed = emit_attention_fp8.seed
                for p in range(2):
                    o8 = oppool.tile([128, 2, 512], F32, tag="o",
                                     name=f"o8_{p}_{j}")
                    for a in range(na):
                        pt = p8pool.tile([128, 2, 1024], FP8, tag="p8",
                                         name=f"p_{j}_{a}_{p}")
                        pt_virgin = p8_seed[0] < 4
                        p8_seed[0] += 1
                        for s in range(2):
                            k = 2 * a + s
                            s_ps = spool.tile([128, 1024], F32, tag="s",
                                              name=f"s_{j}_{k}_{p}")
                            m = emit_s_tile(j, p, k, s_ps)
                            c0 = 0 if pt_virgin else s_c0[0]
                            if c0 == 0:
                                nc.scalar.activation(
                                    pt[:, s, :], s_ps[:],
                                    mybir.ActivationFunctionType.Exp)
                            else:
                                for e in range(2):
                                    nc.scalar.activation(
                                        pt[:, s, 512 * e + c0:512 * (e + 1)],
                                        s_ps[:, 512 * e + c0:512 * (e + 1)],
                                        mybir.ActivationFunctionType.Exp)
                            if m >= 0:
                                w = 128 * m + 128
                                for e in range(2):
                                    nc.vector.tensor_mul(
                                        pt[:, s, 512 * e:512 * e + w],
                                        pt[:, s, 512 * e:512 * e + w],
                                        mn_s[:, 896 - w:896])
                            emit_filler(per_iter)
                        for e in range(2):
                            nc.tensor.matmul(
                                o8[:, e, :],
                                v8_s[:, a, :, 2 * p + e, :],
                                pt[:, :, 512 * e:512 * (e + 1)],
                                start=(a == 0), stop=(a == na - 1),
                                perf_mode=DR,
                                tile_position=(0, 0))
                    onorm.append(normalize(j, p, o8))
                return onorm

            for th in qkv_group_thunks(0):
                th()
            emit_late_dmas()
            for j in range(NQ):
                filler.extend(qkv_group_thunks(j + 1) if j + 1 < NQ else [])
                onorm = emit_attention_j0() if j == 0 else emit_attention_fp8(j)
                filler.extend(proj_group_thunks(j, onorm))
            flush_filler()

    nc.compile()
    return nc


def _get_nc():
    if "nc" not in _CACHE:
        _CACHE["nc"] = _build()
    return _CACHE["nc"]


def _masks():
    """Master causal masks, sliced per diagonal tile at emission time.

    mo/mn [128, 896]: mask[kk, z] = (z >= 768 + kk).  The w-wide suffix
    slice gives [stale-zero prefix | 128-col triangle] for tile offset m:
    col >= 128m + kk  <=>  z = col + 896 - w >= 768 + kk  (w = 128m+128).
    """
    if "mo" not in _CACHE:
        kk = np.arange(128)[:, None]
        z = np.arange(896)[None, :]
        m = (z >= 768 + kk)
        _CACHE["mo"] = m.astype(ml_dtypes.bfloat16)
        _CACHE["mn"] = m.astype(ml_dtypes.float8_e4m3)
    return _CACHE["mo"], _CACHE["mn"]


def kernel(x, Wq, bq, Wk, bk, Wv, bv, Wo, bo):
    x = np.asarray(x, np.float32)
    Wq, bq = np.asarray(Wq, np.float32), np.asarray(bq, np.float32)
    Wk, bk = np.asarray(Wk, np.float32), np.asarray(bk, np.float32)
    Wv, bv = np.asarray(Wv, np.float32), np.asarray(bv, np.float32)
    Wo, bo = np.asarray(Wo, np.float32), np.asarray(bo, np.float32)

    nc = _get_nc()
    mo, mn = _masks()

    def _wimg(w):  # [1024, 256] -> SBUF image [128, 8*256]
        return np.ascontiguousarray(
            w.reshape(8, 128, HS).transpose(1, 0, 2).reshape(128, 8 * HS)
        ).astype(ml_dtypes.bfloat16)

    in_maps = []
    for core in range(N_CORES):
        b, g = divmod(core, 4)
        sl = slice(HS * g, HS * (g + 1))
        xt = x[b].T.reshape(8, 128, 4, 512).transpose(1, 2, 0, 3)
        wo = Wo[sl, :].reshape(2, 128, C).transpose(1, 0, 2)
        in_maps.append({
            "xt": np.ascontiguousarray(xt.reshape(128, NC8 * T)
                                       ).astype(ml_dtypes.bfloat16),
            "wq": _wimg(Wq[:, sl] * 0.125),
            "wk": _wimg(Wk[:, sl]),
            "wv": _wimg(Wv[:, sl]),
            "wo": np.ascontiguousarray(wo.reshape(128, 2 * C)
                                       ).astype(ml_dtypes.bfloat16),
            "bq": (bq[sl] * 0.125).reshape(2, 128).T.copy(),
            "bk": bk[sl].reshape(2, 128).T.copy(),
            "bv": np.broadcast_to(bv[sl], (128, HS)).reshape(128, 4, 64).copy(),
            "mo": mo,
            "mn": mn,
        })

    res = run_bass_kernel_spmd(nc, in_maps, core_ids=list(range(N_CORES)),
                               **_CACHE.get("run_kwargs", {}))
    _CACHE["last_result"] = res

    y = np.zeros((B, T, C), np.float32)
    for core in range(N_CORES):
        b = core // 4
        y[b] += res.results[core]["y"]
    y += bo
    return y
